# revision 1
# baseline (speedup 1.0000x reference)
"""Cross-attention kernel for 8 trn2 NeuronCores.

Problem: B=2, Lq=Lk=2048, D=1024, H=16, dh=64.
  q/k/v = Linear(x); q,k L2-normalized per head; S = q@k.T * 1/8;
  key-pad mask -> -1e9; softmax; mask-aware renorm; eps-smooth toward
  uniform-over-valid; out = attn@v merged -> out_proj.

Sharding: core c handles batch b=c//4, heads [4*(c%4), 4*(c%4)+4)
(two "head pairs" hp of 2 heads each). Each core computes a partial
output-projection over its 256 head dims; the host sums the 8 partials
(4 per batch) and adds the output bias.

Math notes (equivalences used, all within fp rounding of the reference):
  - logits are bounded (|q̂·k̂|/8 <= 0.125) so softmax max-subtraction is
    skipped; masked logits get an additive -30000 bias inside the exp
    (per-key bias = per-partition bias in the transposed S layout), which
    underflows exp to exactly 0 like the reference's -1e9 path.
  - softmax + mask-zero + renorm == (exp @ v) / rowsum(exp) since masked
    entries are exactly 0.
  - eps smoothing: attn' = 0.9*attn + 0.1*valid/nv, so
    out = 0.9*(P@v)/rs + 0.1*vmean, vmean = (valid/nv)@v. The 0.9 is
    folded into the rowsum matmul (lhsT = 1/0.9), vmean*0.1 is computed
    on the host from v_in/Wv/bv exactly.

Device layouts (partition dim first):
  xT     [d_in=128-chunk, tokens]   (host pre-transposes inputs)
  qT/kT  [128 = 2 heads x 64, tokens]  -> S_T matmuls row-packed per head
  v      [tokens, 256]              -> AV matmuls col-packed per head
  S_T    [k-tile=128, q]            -> exp bias = per-partition pad mask
  O_T    [128 = 2 heads x 64, q]    -> feeds out_proj as lhsT directly
"""

import ml_dtypes
import numpy as np

import concourse.bass as bass
from concourse import bacc
import concourse.mybir as mybir
import concourse.tile as tile
from concourse.bass_utils import run_bass_kernel_spmd

F32 = mybir.dt.float32
BF16 = mybir.dt.bfloat16
AF = mybir.ActivationFunctionType

B, L, D = 2, 2048, 1024
H, DH = 16, 64
HEADS_PER_CORE = 4          # -> 256 dims per core, 2 head-pairs
HPC = HEADS_PER_CORE * DH   # 256
SCALE = 0.125               # 1/sqrt(64) / ATTN_TEMP
EPS_SMOOTH = 0.1
INV09 = 1.0 / (1.0 - EPS_SMOOTH)
MASK_BIAS = -30000.0
N_CORES = 8
KT = L // 128               # 16 k tiles
QC = L // 512               # 4 q chunks
NCH = D // 128              # 8 contraction chunks for projections


def _build_nc():
    nc = bacc.Bacc(None)

    xqT = nc.dram_tensor("xqT", [D, L], BF16, kind="ExternalInput")
    xkT = nc.dram_tensor("xkT", [D, L], BF16, kind="ExternalInput")
    xvT = nc.dram_tensor("xvT", [D, L], BF16, kind="ExternalInput")
    wq_t = nc.dram_tensor("wq_t", [D, HPC], BF16, kind="ExternalInput")
    wk_t = nc.dram_tensor("wk_t", [D, HPC], BF16, kind="ExternalInput")
    wv_t = nc.dram_tensor("wv_t", [D, HPC], BF16, kind="ExternalInput")
    wo_t = nc.dram_tensor("wo_t", [HPC, D], BF16, kind="ExternalInput")
    bq = nc.dram_tensor("bq", [2, 1, 128], BF16, kind="ExternalInput")
    bk = nc.dram_tensor("bk", [2, 1, 128], BF16, kind="ExternalInput")
    bv = nc.dram_tensor("bv", [1, HPC], BF16, kind="ExternalInput")
    mbias = nc.dram_tensor("mbias", [128, KT], F32, kind="ExternalInput")
    vmean = nc.dram_tensor("vmean", [2, 128, 1], F32, kind="ExternalInput")
    partial = nc.dram_tensor("partial", [L, D], F32, kind="ExternalOutput")

    with tile.TileContext(nc) as tc:
        with (
            tc.tile_pool(name="consts", bufs=1) as consts,
            tc.tile_pool(name="wpool", bufs=1) as wpool,
            tc.tile_pool(name="persist", bufs=1) as persist,
            tc.tile_pool(name="xstream", bufs=6) as xstream,
            tc.tile_pool(name="xvstream", bufs=8) as xvstream,
            tc.tile_pool(name="l2pool", bufs=4) as l2pool,
            tc.tile_pool(name="ppool", bufs=3) as ppool,
            tc.tile_pool(name="normpool", bufs=4) as normpool,
        ):
            # ---- constants ----
            ones_row = consts.tile([1, 512], BF16, tag="ones_row")
            nc.vector.memset(ones_row, 1.0)
            ones09 = consts.tile([128, 64], BF16, tag="ones09")
            nc.vector.memset(ones09, 1.0)
            blockdiag = consts.tile([128, 128], BF16, tag="blockdiag")
            nc.vector.memset(blockdiag, 0.0)
            nc.vector.memset(blockdiag[0:64, 0:64], 1.0)
            nc.vector.memset(blockdiag[64:128, 64:128], 1.0)
            mbias_sb = consts.tile([128, KT], F32, tag="mbias")
            nc.sync.dma_start(out=mbias_sb, in_=mbias[:, :])
            vmean_sb = []
            for hp in range(2):
                t = consts.tile([128, 1], F32, tag=f"vmean{hp}")
                nc.sync.dma_start(out=t, in_=vmean[hp])
                vmean_sb.append(t)
            bias_sb = {}
            for name, hnd in (("q", bq), ("k", bk)):
                for hp in range(2):
                    t = consts.tile([1, 128], BF16, tag=f"b{name}{hp}")
                    nc.sync.dma_start(out=t, in_=hnd[hp])
                    bias_sb[(name, hp)] = t
            bv_sb = consts.tile([1, HPC], BF16, tag="bv")
            nc.sync.dma_start(out=bv_sb, in_=bv[:, :])

            # ---- weights ----
            # w*_t [D, 256] -> [128, chunk, 256]
            w_sb = {}
            for name, hnd in (("q", wq_t), ("k", wk_t), ("v", wv_t)):
                t = wpool.tile([128, NCH, HPC], BF16, tag=f"w{name}")
                nc.sync.dma_start(
                    out=t, in_=hnd.rearrange("(c p) m -> p c m", p=128)
                )
                w_sb[name] = t
            wo_sb = wpool.tile([128, 2, D], BF16, tag="wo")
            nc.sync.dma_start(
                out=wo_sb, in_=wo_t.rearrange("(h p) m -> p h m", p=128)
            )

            # ---- persistent activations ----
            qTn = [persist.tile([128, L], BF16, tag=f"qTn{hp}", name=f"qTn{hp}")
                   for hp in range(2)]
            kTn = [persist.tile([128, L], BF16, tag=f"kTn{hp}", name=f"kTn{hp}")
                   for hp in range(2)]
            v_sb = persist.tile([128, KT, HPC], BF16, tag="v_sb")
            ofin = [persist.tile([128, L], BF16, tag=f"ofin{hp}", name=f"ofin{hp}")
                    for hp in range(2)]

            # ---- projections ----
            with (
                tc.tile_pool(name="ps_proj", bufs=4, space="PSUM") as ps_proj,
                tc.tile_pool(name="ps_n2", bufs=2, space="PSUM") as ps_n2,
            ):
                # q/k: qT[dout, t] accumulated over d_in chunks
                for name, xhnd, dst in (("q", xqT, qTn), ("k", xkT, kTn)):
                    for qc in range(QC):
                        psums = [
                            ps_proj.tile([128, 512], F32, tag="proj",
                                         name=f"proj{i}")
                            for i in range(2)
                        ]
                        for c in range(NCH):
                            xt = xstream.tile([128, 512], BF16, tag="xt")
                            nc.sync.dma_start(
                                out=xt,
                                in_=xhnd[c * 128:(c + 1) * 128,
                                         qc * 512:(qc + 1) * 512],
                            )
                            for hp in range(2):
                                nc.tensor.matmul(
                                    psums[hp],
                                    lhsT=w_sb[name][:, c, hp * 128:(hp + 1) * 128],
                                    rhs=xt,
                                    start=(c == 0),
                                    stop=False,
                                )
                        for hp in range(2):
                            # + bias (broadcast along tokens via K=1 matmul)
                            nc.tensor.matmul(
                                psums[hp],
                                lhsT=bias_sb[(name, hp)],
                                rhs=ones_row,
                                start=False,
                                stop=True,
                            )
                            # L2 norm over each head's 64 dims
                            sq = l2pool.tile([128, 512], BF16, tag="sq")
                            nc.scalar.square(sq, psums[hp])
                            n2 = ps_n2.tile([128, 512], F32, tag="n2")
                            nc.tensor.matmul(
                                n2, lhsT=blockdiag, rhs=sq, start=True, stop=True
                            )
                            nlen = l2pool.tile([128, 512], F32, tag="nlen")
                            nc.scalar.activation(nlen, n2, AF.Sqrt)
                            rnorm = l2pool.tile([128, 512], F32, tag="rnorm")
                            nc.vector.reciprocal_approx_fast(rnorm, nlen)
                            nc.vector.tensor_mul(
                                dst[hp][:, qc * 512:(qc + 1) * 512],
                                psums[hp], rnorm,
                            )

                # v: v[t, dout] = sum_c xT[c][:, t].T @ w[c]
                for tt in range(KT):
                    vp = ps_proj.tile([128, HPC], F32, tag="proj")
                    for c in range(NCH):
                        xt = xvstream.tile([128, 128], BF16, tag="xvt")
                        nc.sync.dma_start(
                            out=xt,
                            in_=xvT[c * 128:(c + 1) * 128,
                                    tt * 128:(tt + 1) * 128],
                        )
                        nc.tensor.matmul(
                            vp, lhsT=xt, rhs=w_sb["v"][:, c, :],
                            start=(c == 0), stop=False,
                        )
                    nc.tensor.matmul(
                        vp, lhsT=ones_row[:, 0:128], rhs=bv_sb,
                        start=False, stop=True,
                    )
                    nc.vector.tensor_copy(v_sb[:, tt, :], vp)

            # ---- attention ----
            with (
                tc.tile_pool(name="ps_S", bufs=2, space="PSUM") as ps_S,
                tc.tile_pool(name="ps_O", bufs=2, space="PSUM") as ps_O,
                tc.tile_pool(name="ps_rs", bufs=2, space="PSUM") as ps_rs,
            ):
                for hp in range(2):
                    for qc in range(QC):
                        qsl = slice(qc * 512, (qc + 1) * 512)
                        o_ps = ps_O.tile([128, 512], F32, tag="o")
                        rs_ps = ps_rs.tile([128, 512], F32, tag="rs")
                        for kt in range(KT):
                            s_ps = ps_S.tile([128, 1024], F32, tag="s")
                            ksl = slice(kt * 128, (kt + 1) * 128)
                            # S_T = k̂.T q̂ per head, row-packed (K=64 each)
                            nc.tensor.matmul(
                                s_ps[:, 0:512],
                                lhsT=kTn[hp][0:64, ksl],
                                rhs=qTn[hp][0:64, qsl],
                                start=True, stop=True,
                            )
                            nc.tensor.matmul(
                                s_ps[:, 512:1024],
                                lhsT=kTn[hp][64:128, ksl],
                                rhs=qTn[hp][64:128, qsl],
                                start=True, stop=True,
                            )
                            # P = exp(SCALE*S + pad_bias); masked keys -> 0
                            p_sb = ppool.tile([128, 1024], BF16, tag="p")
                            nc.scalar.activation(
                                p_sb, s_ps, AF.Exp,
                                bias=mbias_sb[:, kt:kt + 1], scale=SCALE,
                            )
                            # O_T += v.T @ P, col-packed per head
                            nc.tensor.matmul(
                                o_ps[0:64, :],
                                lhsT=v_sb[:, kt, hp * 128:hp * 128 + 64],
                                rhs=p_sb[:, 0:512],
                                start=(kt == 0), stop=(kt == KT - 1),
                            )
                            nc.tensor.matmul(
                                o_ps[64:128, :],
                                lhsT=v_sb[:, kt, hp * 128 + 64:hp * 128 + 128],
                                rhs=p_sb[:, 512:1024],
                                start=(kt == 0), stop=(kt == KT - 1),
                            )
                            # rs += (1/0.9)*colsum(P), replicated over 64 parts
                            nc.tensor.matmul(
                                rs_ps[0:64, :],
                                lhsT=ones09,
                                rhs=p_sb[:, 0:512],
                                start=(kt == 0), stop=(kt == KT - 1),
                            )
                            nc.tensor.matmul(
                                rs_ps[64:128, :],
                                lhsT=ones09,
                                rhs=p_sb[:, 512:1024],
                                start=(kt == 0), stop=(kt == KT - 1),
                            )
                        # O_final = 0.9*O_T/rs + 0.1*vmean
                        rsb = normpool.tile([128, 512], F32, tag="rsb")
                        nc.vector.tensor_copy(rsb, rs_ps)
                        rr = normpool.tile([128, 512], F32, tag="rr")
                        nc.vector.reciprocal_approx_fast(rr, rsb)
                        om = normpool.tile([128, 512], F32, tag="om")
                        nc.vector.tensor_mul(om, o_ps, rr)
                        nc.vector.tensor_scalar(
                            ofin[hp][:, qsl], om, 1.0 - EPS_SMOOTH,
                            vmean_sb[hp], mybir.AluOpType.mult,
                            mybir.AluOpType.add,
                        )

            # ---- partial output projection ----
            with tc.tile_pool(name="ps_out", bufs=3, space="PSUM") as ps_out:
                for tt in range(KT):
                    tsl = slice(tt * 128, (tt + 1) * 128)
                    for nh in range(2):
                        nsl = slice(nh * 512, (nh + 1) * 512)
                        op = ps_out.tile([128, 512], F32, tag="oproj")
                        nc.tensor.matmul(
                            op, lhsT=ofin[0][:, tsl], rhs=wo_sb[:, 0, nsl],
                            start=True, stop=False,
                        )
                        nc.tensor.matmul(
                            op, lhsT=ofin[1][:, tsl], rhs=wo_sb[:, 1, nsl],
                            start=False, stop=True,
                        )
                        ost = normpool.tile([128, 512], F32, tag="ost")
                        nc.vector.tensor_copy(ost, op)
                        nc.sync.dma_start(out=partial[tsl, nsl], in_=ost)

    nc.finalize()
    return nc


_NC_CACHE = None


def _get_nc():
    global _NC_CACHE
    if _NC_CACHE is None:
        _NC_CACHE = _build_nc()
    return _NC_CACHE


def kernel(q_in, k_in, v_in, kv_pad_mask, Wq, bq, Wk, bk, Wv, bv, Wo, bo,
           _trace=False):
    f32 = np.float32
    q_in = np.asarray(q_in, f32)
    k_in = np.asarray(k_in, f32)
    v_in = np.asarray(v_in, f32)
    mask = np.asarray(kv_pad_mask, bool)
    Wq, bq, Wk, bk, Wv, bv, Wo, bo = (
        np.asarray(a, f32) for a in (Wq, bq, Wk, bk, Wv, bv, Wo, bo)
    )

    nc = _get_nc()

    # per-batch host prep
    xT = {}
    mb = {}
    for b in range(B):
        bf = ml_dtypes.bfloat16
        xT[("q", b)] = np.ascontiguousarray(q_in[b].T).astype(bf)
        xT[("k", b)] = np.ascontiguousarray(k_in[b].T).astype(bf)
        xT[("v", b)] = np.ascontiguousarray(v_in[b].T).astype(bf)
        mb[b] = np.ascontiguousarray(
            np.where(mask[b], MASK_BIAS, 0.0).astype(f32).reshape(KT, 128).T
        )

    in_maps = []
    for core in range(N_CORES):
        b = core // 4
        h0 = (core % 4) * HEADS_PER_CORE
        rows = slice(h0 * DH, h0 * DH + HPC)
        valid = (~mask[b]).astype(f32)
        nv = max(float(valid.sum()), 1.0)
        vscaled = valid * (EPS_SMOOTH / nv)
        # 0.1 * mean_over_valid(v) for this core's 256 dims
        vm = (vscaled @ v_in[b]) @ Wv[rows].T + EPS_SMOOTH * bv[rows]
        in_maps.append({
            "xqT": xT[("q", b)],
            "xkT": xT[("k", b)],
            "xvT": xT[("v", b)],
            "wq_t": np.ascontiguousarray(Wq[rows].T).astype(ml_dtypes.bfloat16),
            "wk_t": np.ascontiguousarray(Wk[rows].T).astype(ml_dtypes.bfloat16),
            "wv_t": np.ascontiguousarray(Wv[rows].T).astype(ml_dtypes.bfloat16),
            "wo_t": np.ascontiguousarray(Wo[:, rows].T).astype(ml_dtypes.bfloat16),
            "bq": np.ascontiguousarray(bq[rows].reshape(2, 1, 128)).astype(ml_dtypes.bfloat16),
            "bk": np.ascontiguousarray(bk[rows].reshape(2, 1, 128)).astype(ml_dtypes.bfloat16),
            "bv": np.ascontiguousarray(bv[rows].reshape(1, HPC)).astype(ml_dtypes.bfloat16),
            "mbias": mb[b],
            "vmean": np.ascontiguousarray(vm.astype(f32).reshape(2, 128, 1)),
        })

    res = run_bass_kernel_spmd(nc, in_maps, core_ids=list(range(N_CORES)),
                               trace=_trace)
    out = np.zeros((B, L, D), f32)
    for core in range(N_CORES):
        out[core // 4] += res.results[core]["partial"]
    out += bo[None, None, :]
    if _trace:
        kernel._last_result = res
    return out



# revision 3
# speedup vs baseline: 1.8448x; 1.8448x over previous
"""Cross-attention kernel for 8 trn2 NeuronCores.

Problem: B=2, Lq=Lk=2048, D=1024, H=16, dh=64.
  q/k/v = Linear(x); q,k L2-normalized per head; S = q@k.T * 1/8;
  key-pad mask -> -1e9; softmax; mask-aware renorm; eps-smooth toward
  uniform-over-valid; out = attn@v merged -> out_proj.

Sharding: core c handles batch b=c//4, heads [4*(c%4), 4*(c%4)+4)
(two "head pairs" hp of 2 heads each). Each core computes a partial
output-projection over its 256 head dims; the host sums the 8 partials
(4 per batch) and adds a combined bias.

Key optimizations vs the naive formulation:
  - Key compaction: masked keys contribute exactly 0 after the
    reference's mask+renorm, so the host gathers only the valid keys
    (padded to a 128 multiple). Halves k/v projection, S, exp, AV work.
  - exp(SCALE*S - 30000) == 0 exactly for the pad keys in the last
    tile(s) (per-partition bias in the transposed S layout).
  - Softmax denominator comes free from the AV matmul: lhsT is
    [v_head | ones] (head A) / [ones | v_head] (head B), so the same
    instruction yields O on one 64-partition half and the replicated
    rowsum on the other. No separate rowsum matmuls.
  - 0.9 eps-smoothing factor is folded into Wo on the host; the
    0.1*uniform term is token-independent after out_proj and is folded
    into a host-side per-batch constant.
  - q/k projections run as fp8e4 DoubleRow matmuls (2 contraction
    tiles per instruction, 2x PE throughput). The fp8 quantization
    error washes out through L2-norm + softmax (<2e-4 on the output).

Device layouts (partition dim first):
  xT     [d_in chunk, tokens]      (host pre-transposes inputs)
  qT/kT  [128 = 2 heads x 64, tokens]
  v2     [tokens, kt, 192] = [vA(64) | ones(64) | vB(64)] per kt
  S_T    [k-tile=128, q]           exp bias = per-partition pad mask
  O      [128, q] = [O_A | rs_A] and [rs_B | O_B] per head pair
"""

import ml_dtypes
import numpy as np

from concourse import bacc
import concourse.mybir as mybir
import concourse.tile as tile
from concourse.bass_utils import run_bass_kernel_spmd

F32 = mybir.dt.float32
BF16 = mybir.dt.bfloat16
FP8 = mybir.dt.float8e4
AF = mybir.ActivationFunctionType
DR = mybir.MatmulPerfMode.DoubleRow

B, L, D = 2, 2048, 1024
H, DH = 16, 64
HEADS_PER_CORE = 4          # -> 256 dims per core, 2 head-pairs
HPC = HEADS_PER_CORE * DH   # 256
SCALE = 0.125               # 1/sqrt(64) / ATTN_TEMP
EPS_SMOOTH = 0.1
MASK_BIAS = -30000.0
N_CORES = 8
QC = L // 512               # 4 q chunks
NCH = D // 128              # 8 contraction chunks for projections


def _chunks(total, width):
    off = 0
    out = []
    while off < total:
        w = min(width, total - off)
        out.append((off, w))
        off += w
    return out


def _build_nc(kt_k):
    KP = kt_k * 128
    nc = bacc.Bacc(None)

    xqT = nc.dram_tensor("xqT", [D, L], FP8, kind="ExternalInput")
    xkT = nc.dram_tensor("xkT", [D, KP], FP8, kind="ExternalInput")
    xvT = nc.dram_tensor("xvT", [D, KP], BF16, kind="ExternalInput")
    wq8 = nc.dram_tensor("wq8", [D, HPC], FP8, kind="ExternalInput")
    wk8 = nc.dram_tensor("wk8", [D, HPC], FP8, kind="ExternalInput")
    wv_t = nc.dram_tensor("wv_t", [D, HPC], BF16, kind="ExternalInput")
    wo_t = nc.dram_tensor("wo_t", [HPC, D], BF16, kind="ExternalInput")
    bq = nc.dram_tensor("bq", [2, 1, 128], BF16, kind="ExternalInput")
    bk = nc.dram_tensor("bk", [2, 1, 128], BF16, kind="ExternalInput")
    bv = nc.dram_tensor("bv", [1, HPC], BF16, kind="ExternalInput")
    mbias = nc.dram_tensor("mbias", [128, kt_k], F32, kind="ExternalInput")
    partial = nc.dram_tensor("partial", [L, D], F32, kind="ExternalOutput")

    with tile.TileContext(nc) as tc:
        with (
            tc.tile_pool(name="consts", bufs=1) as consts,
            tc.tile_pool(name="wpool", bufs=1) as wpool,
            tc.tile_pool(name="persist", bufs=1) as persist,
            tc.tile_pool(name="xstream", bufs=6) as xstream,
            tc.tile_pool(name="xvstream", bufs=4) as xvstream,
            tc.tile_pool(name="l2pool", bufs=4) as l2pool,
            tc.tile_pool(name="ppool", bufs=4) as ppool,
            tc.tile_pool(name="dpool", bufs=2) as dpool,
            tc.tile_pool(name="opool", bufs=4) as opool,
        ):
            # ---- constants ----
            ones_row = consts.tile([1, 512], BF16, tag="ones_row")
            nc.vector.memset(ones_row, 1.0)
            blockdiag = consts.tile([128, 128], BF16, tag="blockdiag")
            nc.vector.memset(blockdiag, 0.0)
            nc.vector.memset(blockdiag[0:64, 0:64], 1.0)
            nc.vector.memset(blockdiag[64:128, 64:128], 1.0)
            mbias_sb = consts.tile([128, kt_k], F32, tag="mbias")
            nc.sync.dma_start(out=mbias_sb, in_=mbias[:, :])
            bias_sb = {}
            for name, hnd in (("q", bq), ("k", bk)):
                for hp in range(2):
                    t = consts.tile([1, 128], BF16, tag=f"b{name}{hp}")
                    nc.sync.dma_start(out=t, in_=hnd[hp])
                    bias_sb[(name, hp)] = t
            bv_sb = consts.tile([1, HPC], BF16, tag="bv")
            nc.sync.dma_start(out=bv_sb, in_=bv[:, :])

            # ---- weights ----
            # q/k: fp8 DoubleRow layout [128, chunk-pair, 2, 256]
            w8 = {}
            for name, hnd in (("q", wq8), ("k", wk8)):
                t = wpool.tile([128, NCH // 2, 2, HPC], FP8, tag=f"w8{name}")
                nc.sync.dma_start(
                    out=t, in_=hnd.rearrange("(c j p) m -> p c j m", p=128, j=2)
                )
                w8[name] = t
            wv_sb = wpool.tile([128, NCH, HPC], BF16, tag="wv")
            nc.sync.dma_start(
                out=wv_sb, in_=wv_t.rearrange("(c p) m -> p c m", p=128)
            )
            wo_sb = wpool.tile([128, 2, D], BF16, tag="wo")
            nc.sync.dma_start(
                out=wo_sb, in_=wo_t.rearrange("(h p) m -> p h m", p=128)
            )

            # ---- persistent activations ----
            qTn = [persist.tile([128, L], BF16, tag=f"qTn{hp}", name=f"qTn{hp}")
                   for hp in range(2)]
            kTn = [persist.tile([128, KP], BF16, tag=f"kTn{hp}", name=f"kTn{hp}")
                   for hp in range(2)]
            # [vA | ones | ... | vB] per k-tile; ones come from the memset
            v2 = [persist.tile([128, kt_k, 192], BF16, tag=f"v2{hp}",
                               name=f"v2{hp}") for hp in range(2)]
            for hp in range(2):
                nc.gpsimd.memset(v2[hp], 1.0)
            ofin = [persist.tile([128, L], BF16, tag=f"ofin{hp}", name=f"ofin{hp}")
                    for hp in range(2)]

            # ---- phase A: projections ----
            with (
                tc.tile_pool(name="ps_proj", bufs=4, space="PSUM") as ps_proj,
                tc.tile_pool(name="ps_n2", bufs=2, space="PSUM") as ps_n2,
                tc.tile_pool(name="ps_vp", bufs=2, space="PSUM") as ps_vp,
            ):
                def qk_proj(name, xhnd, dst, chunks):
                    for off, W in chunks:
                        psums = [
                            ps_proj.tile([128, 512], F32, tag="proj",
                                         name=f"proj{i}")
                            for i in range(2)
                        ]
                        for c in range(NCH // 2):
                            xt = xstream.tile([128, 2, 512], FP8, tag="xt")
                            nc.sync.dma_start(
                                out=xt[:, :, 0:W],
                                in_=xhnd[c * 256:(c + 1) * 256,
                                         off:off + W].rearrange(
                                    "(j p) n -> p j n", p=128),
                            )
                            for hp in range(2):
                                nc.tensor.matmul(
                                    psums[hp][:, 0:W],
                                    lhsT=w8[name][:, c, :,
                                                  hp * 128:(hp + 1) * 128],
                                    rhs=xt[:, :, 0:W],
                                    start=(c == 0),
                                    stop=False,
                                    perf_mode=DR,
                                )
                        for hp in range(2):
                            # + bias (broadcast along tokens via K=1 matmul)
                            nc.tensor.matmul(
                                psums[hp][:, 0:W],
                                lhsT=bias_sb[(name, hp)],
                                rhs=ones_row[:, 0:W],
                                start=False,
                                stop=True,
                            )
                            # L2 norm over each head's 64 dims
                            sq = l2pool.tile([128, 512], BF16, tag="sq")
                            nc.scalar.square(sq[:, 0:W], psums[hp][:, 0:W])
                            n2 = ps_n2.tile([128, 512], F32, tag="n2")
                            nc.tensor.matmul(
                                n2[:, 0:W], lhsT=blockdiag, rhs=sq[:, 0:W],
                                start=True, stop=True,
                            )
                            nlen = l2pool.tile([128, 512], F32, tag="nlen")
                            nc.scalar.activation(nlen[:, 0:W], n2[:, 0:W],
                                                 AF.Sqrt)
                            rnorm = l2pool.tile([128, 512], F32, tag="rnorm")
                            nc.vector.reciprocal_approx_fast(rnorm[:, 0:W],
                                                             nlen[:, 0:W])
                            nc.vector.tensor_mul(
                                dst[hp][:, off:off + W],
                                psums[hp][:, 0:W], rnorm[:, 0:W],
                            )

                qk_proj("k", xkT, kTn, _chunks(KP, 512))

                # v: v[t, dout]; interleave token-tile pairs to avoid
                # serial psum-accumulation stalls
                for tp in range(0, kt_k, 2):
                    tts = [tp] + ([tp + 1] if tp + 1 < kt_k else [])
                    nt = len(tts)
                    vps = [ps_vp.tile([128, HPC], F32, tag="vp",
                                      name=f"vp{i}") for i in range(nt)]
                    for c in range(NCH):
                        xvt = xvstream.tile([128, 256], BF16, tag="xvt")
                        nc.sync.dma_start(
                            out=xvt[:, 0:128 * nt],
                            in_=xvT[c * 128:(c + 1) * 128,
                                    tp * 128:tp * 128 + 128 * nt],
                        )
                        for i in range(nt):
                            nc.tensor.matmul(
                                vps[i], lhsT=xvt[:, i * 128:(i + 1) * 128],
                                rhs=wv_sb[:, c, :],
                                start=(c == 0), stop=False,
                            )
                    for i, tt in enumerate(tts):
                        nc.tensor.matmul(
                            vps[i], lhsT=ones_row[:, 0:128], rhs=bv_sb,
                            start=False, stop=True,
                        )
                        for hp in range(2):
                            nc.vector.tensor_copy(
                                v2[hp][:, tt, 0:64],
                                vps[i][:, hp * 128:hp * 128 + 64])
                            nc.vector.tensor_copy(
                                v2[hp][:, tt, 128:192],
                                vps[i][:, hp * 128 + 64:hp * 128 + 128])

                qk_proj("q", xqT, qTn, _chunks(L, 512))

            # ---- phase B: attention ----
            with (
                tc.tile_pool(name="ps_S", bufs=2, space="PSUM") as ps_S,
                tc.tile_pool(name="ps_O", bufs=3, space="PSUM") as ps_O,
            ):
                for qc in range(QC):
                    qsl = slice(qc * 512, (qc + 1) * 512)
                    for hp in range(2):
                        oA = ps_O.tile([128, 512], F32, tag="o", name="oA")
                        oB = ps_O.tile([128, 512], F32, tag="o", name="oB")
                        for kt in range(kt_k):
                            ksl = slice(kt * 128, (kt + 1) * 128)
                            s_ps = ps_S.tile([128, 1024], F32, tag="s")
                            # S_T = k̂.T q̂ per head, row-packed (K=64 each)
                            nc.tensor.matmul(
                                s_ps[:, 0:512],
                                lhsT=kTn[hp][0:64, ksl],
                                rhs=qTn[hp][0:64, qsl],
                                start=True, stop=True,
                            )
                            nc.tensor.matmul(
                                s_ps[:, 512:1024],
                                lhsT=kTn[hp][64:128, ksl],
                                rhs=qTn[hp][64:128, qsl],
                                start=True, stop=True,
                            )
                            # P = exp(SCALE*S + pad_bias); pad keys -> 0
                            p_sb = ppool.tile([128, 1024], BF16, tag="p")
                            nc.scalar.activation(
                                p_sb, s_ps, AF.Exp,
                                bias=mbias_sb[:, kt:kt + 1], scale=SCALE,
                            )
                            # O += [v|1].T @ P : O_A on parts 0:64 + rs_A
                            # replicated on 64:128 (head B mirrored)
                            nc.tensor.matmul(
                                oA,
                                lhsT=v2[hp][:, kt, 0:128],
                                rhs=p_sb[:, 0:512],
                                start=(kt == 0), stop=(kt == kt_k - 1),
                            )
                            nc.tensor.matmul(
                                oB,
                                lhsT=v2[hp][:, kt, 64:192],
                                rhs=p_sb[:, 512:1024],
                                start=(kt == 0), stop=(kt == kt_k - 1),
                            )
                        # division: ofin = O / rs (0.9 folded into Wo)
                        rspack = dpool.tile([128, 512], F32, tag="rspack")
                        nc.vector.tensor_copy(rspack[64:128, :], oA[64:128, :])
                        nc.vector.tensor_copy(rspack[0:64, :], oB[0:64, :])
                        rr = dpool.tile([128, 512], F32, tag="rr")
                        nc.vector.reciprocal_approx_fast(rr, rspack)
                        rsw = dpool.tile([128, 512], F32, tag="rsw")
                        nc.sync.dma_start(out=rsw[0:64, :], in_=rr[64:128, :])
                        nc.sync.dma_start(out=rsw[64:128, :], in_=rr[0:64, :])
                        nc.vector.tensor_mul(
                            ofin[hp][0:64, qsl], oA[0:64, :], rsw[0:64, :])
                        nc.vector.tensor_mul(
                            ofin[hp][64:128, qsl], oB[64:128, :],
                            rsw[64:128, :])

            # ---- phase C: partial output projection ----
            with tc.tile_pool(name="ps_out", bufs=4, space="PSUM") as ps_out:
                for tt in range(L // 128):
                    tsl = slice(tt * 128, (tt + 1) * 128)
                    for nh in range(2):
                        nsl = slice(nh * 512, (nh + 1) * 512)
                        op = ps_out.tile([128, 512], F32, tag="oproj")
                        nc.tensor.matmul(
                            op, lhsT=ofin[0][:, tsl], rhs=wo_sb[:, 0, nsl],
                            start=True, stop=False,
                        )
                        nc.tensor.matmul(
                            op, lhsT=ofin[1][:, tsl], rhs=wo_sb[:, 1, nsl],
                            start=False, stop=True,
                        )
                        ost = opool.tile([128, 512], F32, tag="ost")
                        if (tt * 2 + nh) % 2 == 0:
                            nc.vector.tensor_copy(ost, op)
                        else:
                            nc.scalar.copy(ost, op)
                        nc.sync.dma_start(out=partial[tsl, nsl], in_=ost)

    nc.finalize()
    return nc


_NC_CACHE = {}


def _get_nc(kt_k):
    if kt_k not in _NC_CACHE:
        _NC_CACHE[kt_k] = _build_nc(kt_k)
    return _NC_CACHE[kt_k]


def kernel(q_in, k_in, v_in, kv_pad_mask, Wq, bq, Wk, bk, Wv, bv, Wo, bo,
           _trace=False):
    f32 = np.float32
    bf = ml_dtypes.bfloat16
    f8 = ml_dtypes.float8_e4m3fn
    q_in = np.asarray(q_in, f32)
    k_in = np.asarray(k_in, f32)
    v_in = np.asarray(v_in, f32)
    mask = np.asarray(kv_pad_mask, bool)
    Wq, bq, Wk, bk, Wv, bv, Wo, bo = (
        np.asarray(a, f32) for a in (Wq, bq, Wk, bk, Wv, bv, Wo, bo)
    )

    idx = [np.flatnonzero(~mask[b]) for b in range(B)]
    nv = [len(i) for i in idx]
    kt_k = max(1, max((n + 127) // 128 for n in nv))
    KP = kt_k * 128
    nc = _get_nc(kt_k)

    # per-batch host prep
    xq8, xk8, xvT, mb, hostconst = {}, {}, {}, {}, {}
    for b in range(B):
        xq8[b] = np.ascontiguousarray(q_in[b].T).astype(f8)
        kc = np.zeros((KP, D), f32)
        kc[:nv[b]] = k_in[b][idx[b]]
        xk8[b] = np.ascontiguousarray(kc.T).astype(f8)
        vc = np.zeros((KP, D), f32)
        vc[:nv[b]] = v_in[b][idx[b]]
        xvT[b] = np.ascontiguousarray(vc.T).astype(bf)
        mrow = np.where(np.arange(KP) < nv[b], 0.0, MASK_BIAS).astype(f32)
        mb[b] = np.ascontiguousarray(mrow.reshape(kt_k, 128).T)
        # 0.1 * uniform-over-valid term is token-independent after out_proj
        n = max(float(nv[b]), 1.0)
        valid = (~mask[b]).astype(f32) / n
        vmean_full = (valid @ v_in[b]) @ Wv.T + bv
        hostconst[b] = bo + EPS_SMOOTH * (vmean_full @ Wo.T)

    in_maps = []
    for core in range(N_CORES):
        b = core // 4
        h0 = (core % 4) * HEADS_PER_CORE
        rows = slice(h0 * DH, h0 * DH + HPC)
        in_maps.append({
            "xqT": xq8[b],
            "xkT": xk8[b],
            "xvT": xvT[b],
            "wq8": np.ascontiguousarray(Wq[rows].T).astype(f8),
            "wk8": np.ascontiguousarray(Wk[rows].T).astype(f8),
            "wv_t": np.ascontiguousarray(Wv[rows].T).astype(bf),
            "wo_t": np.ascontiguousarray(
                (1.0 - EPS_SMOOTH) * Wo[:, rows].T).astype(bf),
            "bq": np.ascontiguousarray(bq[rows].reshape(2, 1, 128)).astype(bf),
            "bk": np.ascontiguousarray(bk[rows].reshape(2, 1, 128)).astype(bf),
            "bv": np.ascontiguousarray(bv[rows].reshape(1, HPC)).astype(bf),
            "mbias": mb[b],
        })

    res = run_bass_kernel_spmd(nc, in_maps, core_ids=list(range(N_CORES)),
                               trace=_trace)
    out = np.zeros((B, L, D), f32)
    for core in range(N_CORES):
        out[core // 4] += res.results[core]["partial"]
    for b in range(B):
        out[b] += hostconst[b][None, :]
    if _trace:
        kernel._last_result = res
    return out


# revision 4
# speedup vs baseline: 1.9777x; 1.0720x over previous
"""Cross-attention kernel for 8 trn2 NeuronCores.

Problem: B=2, Lq=Lk=2048, D=1024, H=16, dh=64.
  q/k/v = Linear(x); q,k L2-normalized per head; S = q@k.T * 1/8;
  key-pad mask -> -1e9; softmax; mask-aware renorm; eps-smooth toward
  uniform-over-valid; out = attn@v merged -> out_proj.

Sharding: core c handles batch b=c//4, heads [4*(c%4), 4*(c%4)+4)
(two "head pairs" hp of 2 heads each). Each core computes a partial
output-projection over its 256 head dims; the host sums the 8 partials
(4 per batch) and adds a combined bias.

Key optimizations vs the naive formulation:
  - Key compaction: masked keys contribute exactly 0 after the
    reference's mask+renorm, so the host gathers only the valid keys
    (padded to a 128 multiple). Halves k/v projection, S, exp, AV work.
  - exp(SCALE*S - 30000) == 0 exactly for the pad keys in the last
    tile(s) (per-partition bias in the transposed S layout).
  - Softmax denominator comes free from the AV matmul: lhsT is
    [v_head | ones] (head A) / [ones | v_head] (head B), so the same
    instruction yields O on one 64-partition half and the replicated
    rowsum on the other. No separate rowsum matmuls.
  - 0.9 eps-smoothing factor is folded into Wo on the host; the
    0.1*uniform term is token-independent after out_proj and is folded
    into a host-side per-batch constant.
  - q/k projections run as fp8e4 DoubleRow matmuls (2 contraction
    tiles per instruction, 2x PE throughput). The fp8 quantization
    error washes out through L2-norm + softmax (<2e-4 on the output).

Device layouts (partition dim first):
  xT     [d_in chunk, tokens]      (host pre-transposes inputs)
  qT/kT  [128 = 2 heads x 64, tokens]
  v2     [tokens, kt, 192] = [vA(64) | ones(64) | vB(64)] per kt
  S_T    [k-tile=128, q]           exp bias = per-partition pad mask
  O      [128, q] = [O_A | rs_A] and [rs_B | O_B] per head pair
"""

import ml_dtypes
import numpy as np

from concourse import bacc
import concourse.mybir as mybir
import concourse.tile as tile
from concourse.bass_utils import run_bass_kernel_spmd

F32 = mybir.dt.float32
BF16 = mybir.dt.bfloat16
FP8 = mybir.dt.float8e4
AF = mybir.ActivationFunctionType
DR = mybir.MatmulPerfMode.DoubleRow

B, L, D = 2, 2048, 1024
H, DH = 16, 64
HEADS_PER_CORE = 4          # -> 256 dims per core, 2 head-pairs
HPC = HEADS_PER_CORE * DH   # 256
SCALE = 0.125               # 1/sqrt(64) / ATTN_TEMP
EPS_SMOOTH = 0.1
MASK_BIAS = -30000.0
N_CORES = 8
QC = L // 512               # 4 q chunks
NCH = D // 128              # 8 contraction chunks for projections


def _chunks(total, width):
    off = 0
    out = []
    while off < total:
        w = min(width, total - off)
        out.append((off, w))
        off += w
    return out


def _build_nc(kt_k, kt_full):
    KP = kt_k * 128
    nc = bacc.Bacc(None)

    xqT = nc.dram_tensor("xqT", [D, L], FP8, kind="ExternalInput")
    xkT = nc.dram_tensor("xkT", [D, KP], FP8, kind="ExternalInput")
    xvT = nc.dram_tensor("xvT", [D, KP], BF16, kind="ExternalInput")
    wq8 = nc.dram_tensor("wq8", [D, HPC], FP8, kind="ExternalInput")
    wk8 = nc.dram_tensor("wk8", [D, HPC], FP8, kind="ExternalInput")
    wv_t = nc.dram_tensor("wv_t", [D, HPC], BF16, kind="ExternalInput")
    wo_t = nc.dram_tensor("wo_t", [HPC, D], BF16, kind="ExternalInput")
    bq = nc.dram_tensor("bq", [2, 1, 128], BF16, kind="ExternalInput")
    bk = nc.dram_tensor("bk", [2, 1, 128], BF16, kind="ExternalInput")
    bv = nc.dram_tensor("bv", [1, HPC], BF16, kind="ExternalInput")
    mbias = nc.dram_tensor("mbias", [128, kt_k], F32, kind="ExternalInput")
    partial = nc.dram_tensor("partial", [L, D], BF16, kind="ExternalOutput")

    with tile.TileContext(nc) as tc:
        with (
            tc.tile_pool(name="consts", bufs=1) as consts,
            tc.tile_pool(name="wpool", bufs=1) as wpool,
            tc.tile_pool(name="persist", bufs=1) as persist,
            tc.tile_pool(name="xstream", bufs=6) as xstream,
            tc.tile_pool(name="xvstream", bufs=4) as xvstream,
            tc.tile_pool(name="l2pool", bufs=4) as l2pool,
            tc.tile_pool(name="ppool", bufs=4) as ppool,
            tc.tile_pool(name="dpool", bufs=2) as dpool,
            tc.tile_pool(name="tpool", bufs=3) as tpool,
            tc.tile_pool(name="opool", bufs=4) as opool,
        ):
            # ---- constants ----
            ones_row = consts.tile([1, 512], BF16, tag="ones_row")
            nc.vector.memset(ones_row, 1.0)
            blockdiag = consts.tile([128, 128], BF16, tag="blockdiag")
            nc.vector.memset(blockdiag, 0.0)
            nc.vector.memset(blockdiag[0:64, 0:64], 1.0)
            nc.vector.memset(blockdiag[64:128, 64:128], 1.0)
            mbias_sb = consts.tile([128, kt_k], F32, tag="mbias")
            nc.sync.dma_start(out=mbias_sb, in_=mbias[:, :])
            bias_sb = {}
            for name, hnd in (("q", bq), ("k", bk)):
                for hp in range(2):
                    t = consts.tile([1, 128], BF16, tag=f"b{name}{hp}")
                    nc.sync.dma_start(out=t, in_=hnd[hp])
                    bias_sb[(name, hp)] = t
            bv_sb = consts.tile([1, HPC], BF16, tag="bv")
            nc.sync.dma_start(out=bv_sb, in_=bv[:, :])

            # ---- weights ----
            # q/k: fp8 DoubleRow layout [128, chunk-pair, 2, 256]
            w8 = {}
            for name, hnd in (("q", wq8), ("k", wk8)):
                t = wpool.tile([128, NCH // 2, 2, HPC], FP8, tag=f"w8{name}",
                               name=f"w8{name}")
                w8[name] = t
            nc.sync.dma_start(
                out=w8["k"],
                in_=wk8.rearrange("(c j p) m -> p c j m", p=128, j=2))
            wv_sb = wpool.tile([128, NCH, HPC], BF16, tag="wv")
            wo_sb = wpool.tile([128, 2, D], BF16, tag="wo")

            # ---- persistent activations ----
            qTn = [persist.tile([128, L], BF16, tag=f"qTn{hp}", name=f"qTn{hp}")
                   for hp in range(2)]
            kTn = [persist.tile([128, KP], BF16, tag=f"kTn{hp}", name=f"kTn{hp}")
                   for hp in range(2)]
            # [vA | ones | ... | vB] per k-tile; ones come from the memset
            v2 = [persist.tile([128, kt_k, 192], BF16, tag=f"v2{hp}",
                               name=f"v2{hp}") for hp in range(2)]
            for hp in range(2):
                nc.gpsimd.memset(v2[hp], 1.0)
            ofin = [persist.tile([128, L], BF16, tag=f"ofin{hp}", name=f"ofin{hp}")
                    for hp in range(2)]

            # ---- phase A: projections ----
            with (
                tc.tile_pool(name="ps_proj", bufs=4, space="PSUM") as ps_proj,
                tc.tile_pool(name="ps_n2", bufs=2, space="PSUM") as ps_n2,
                tc.tile_pool(name="ps_vp", bufs=2, space="PSUM") as ps_vp,
            ):
                def qk_proj(name, xhnd, dst, chunks):
                    for off, W in chunks:
                        psums = [
                            ps_proj.tile([128, 512], F32, tag="proj",
                                         name=f"proj{i}")
                            for i in range(2)
                        ]
                        for c in range(NCH // 2):
                            xt = xstream.tile([128, 2, 512], FP8, tag="xt")
                            nc.sync.dma_start(
                                out=xt[:, :, 0:W],
                                in_=xhnd[c * 256:(c + 1) * 256,
                                         off:off + W].rearrange(
                                    "(j p) n -> p j n", p=128),
                            )
                            for hp in range(2):
                                nc.tensor.matmul(
                                    psums[hp][:, 0:W],
                                    lhsT=w8[name][:, c, :,
                                                  hp * 128:(hp + 1) * 128],
                                    rhs=xt[:, :, 0:W],
                                    start=(c == 0),
                                    stop=False,
                                    perf_mode=DR,
                                )
                        for hp in range(2):
                            # + bias (broadcast along tokens via K=1 matmul)
                            nc.tensor.matmul(
                                psums[hp][:, 0:W],
                                lhsT=bias_sb[(name, hp)],
                                rhs=ones_row[:, 0:W],
                                start=False,
                                stop=True,
                            )
                            # L2 norm over each head's 64 dims
                            sq = l2pool.tile([128, 512], BF16, tag="sq")
                            nc.scalar.square(sq[:, 0:W], psums[hp][:, 0:W])
                            n2 = ps_n2.tile([128, 512], F32, tag="n2")
                            nc.tensor.matmul(
                                n2[:, 0:W], lhsT=blockdiag, rhs=sq[:, 0:W],
                                start=True, stop=True,
                            )
                            nlen = l2pool.tile([128, 512], F32, tag="nlen")
                            nc.scalar.activation(nlen[:, 0:W], n2[:, 0:W],
                                                 AF.Sqrt)
                            rnorm = l2pool.tile([128, 512], F32, tag="rnorm")
                            nc.vector.reciprocal_approx_fast(rnorm[:, 0:W],
                                                             nlen[:, 0:W])
                            nc.vector.tensor_mul(
                                dst[hp][:, off:off + W],
                                psums[hp][:, 0:W], rnorm[:, 0:W],
                            )

                qk_proj("k", xkT, kTn, _chunks(KP, 512))

                # deferred weight loads (keeps k-proj off the critical path)
                nc.sync.dma_start(
                    out=w8["q"],
                    in_=wq8.rearrange("(c j p) m -> p c j m", p=128, j=2))
                nc.sync.dma_start(
                    out=wv_sb, in_=wv_t.rearrange("(c p) m -> p c m", p=128))
                nc.sync.dma_start(
                    out=wo_sb, in_=wo_t.rearrange("(h p) m -> p h m", p=128))

                # v: v[t, dout]; interleave token-tile pairs to avoid
                # serial psum-accumulation stalls
                for tp in range(0, kt_k, 2):
                    tts = [tp] + ([tp + 1] if tp + 1 < kt_k else [])
                    nt = len(tts)
                    vps = [ps_vp.tile([128, HPC], F32, tag="vp",
                                      name=f"vp{i}") for i in range(nt)]
                    for c in range(NCH):
                        xvt = xvstream.tile([128, 256], BF16, tag="xvt")
                        nc.sync.dma_start(
                            out=xvt[:, 0:128 * nt],
                            in_=xvT[c * 128:(c + 1) * 128,
                                    tp * 128:tp * 128 + 128 * nt],
                        )
                        for i in range(nt):
                            nc.tensor.matmul(
                                vps[i], lhsT=xvt[:, i * 128:(i + 1) * 128],
                                rhs=wv_sb[:, c, :],
                                start=(c == 0), stop=False,
                            )
                    for i, tt in enumerate(tts):
                        nc.tensor.matmul(
                            vps[i], lhsT=ones_row[:, 0:128], rhs=bv_sb,
                            start=False, stop=True,
                        )
                        for hp in range(2):
                            nc.vector.tensor_copy(
                                v2[hp][:, tt, 0:64],
                                vps[i][:, hp * 128:hp * 128 + 64])
                            nc.vector.tensor_copy(
                                v2[hp][:, tt, 128:192],
                                vps[i][:, hp * 128 + 64:hp * 128 + 128])

                qk_proj("q", xqT, qTn, _chunks(L, 512))

            # ---- phase B: attention ----
            with (
                tc.tile_pool(name="ps_S", bufs=2, space="PSUM") as ps_S,
                tc.tile_pool(name="ps_O", bufs=3, space="PSUM") as ps_O,
            ):
                # exact 2nd-order Taylor of exp (|logit|<=1/8):
                # t = a*s + b, p = t^2 + 0.5 with a=SCALE/sqrt(2), b=1/sqrt(2)
                PA = SCALE / np.sqrt(2.0)
                PB = 1.0 / np.sqrt(2.0)
                poly_kts = {kt for kt in range(kt_full) if kt % 3 == 2}

                def emit_S(hp, qsl, kt):
                    ksl = slice(kt * 128, (kt + 1) * 128)
                    s_ps = ps_S.tile([128, 1024], F32, tag="s")
                    # S_T = k̂.T q̂ per head, row-packed (K=64 each)
                    nc.tensor.matmul(
                        s_ps[:, 0:512],
                        lhsT=kTn[hp][0:64, ksl],
                        rhs=qTn[hp][0:64, qsl],
                        start=True, stop=True,
                    )
                    nc.tensor.matmul(
                        s_ps[:, 512:1024],
                        lhsT=kTn[hp][64:128, ksl],
                        rhs=qTn[hp][64:128, qsl],
                        start=True, stop=True,
                    )
                    return s_ps

                def emit_P(s_ps, kt):
                    # P = exp(SCALE*S + pad_bias); pad keys -> 0
                    p_sb = ppool.tile([128, 1024], BF16, tag="p")
                    if kt in poly_kts:
                        # DVE Taylor path (all-valid tiles only)
                        t1 = tpool.tile([128, 1024], BF16, tag="t1")
                        nc.vector.tensor_scalar(
                            t1, s_ps, PA, PB,
                            mybir.AluOpType.mult, mybir.AluOpType.add)
                        t2 = tpool.tile([128, 1024], BF16, tag="t2")
                        nc.vector.tensor_mul(t2, t1, t1)
                        nc.vector.tensor_scalar_add(p_sb, t2, 0.5)
                    else:
                        nc.scalar.activation(
                            p_sb, s_ps, AF.Exp,
                            bias=mbias_sb[:, kt:kt + 1], scale=SCALE,
                        )
                    return p_sb

                def emit_AV(hp, oA, oB, p_sb, kt):
                    # O += [v|1].T @ P : O_A on parts 0:64 + rs_A
                    # replicated on 64:128 (head B mirrored)
                    nc.tensor.matmul(
                        oA,
                        lhsT=v2[hp][:, kt, 0:128],
                        rhs=p_sb[:, 0:512],
                        start=(kt == 0), stop=(kt == kt_k - 1),
                    )
                    nc.tensor.matmul(
                        oB,
                        lhsT=v2[hp][:, kt, 64:192],
                        rhs=p_sb[:, 512:1024],
                        start=(kt == 0), stop=(kt == kt_k - 1),
                    )

                for qc in range(QC):
                    qsl = slice(qc * 512, (qc + 1) * 512)
                    for hp in range(2):
                        oA = ps_O.tile([128, 512], F32, tag="o", name="oA")
                        oB = ps_O.tile([128, 512], F32, tag="o", name="oB")
                        # software pipeline: S one k-tile ahead of P/AV
                        s_cur = emit_S(hp, qsl, 0)
                        for kt in range(kt_k):
                            p_sb = emit_P(s_cur, kt)
                            if kt + 1 < kt_k:
                                s_cur = emit_S(hp, qsl, kt + 1)
                            emit_AV(hp, oA, oB, p_sb, kt)
                        # division: ofin = O / rs (0.9 folded into Wo)
                        rspack = dpool.tile([128, 512], F32, tag="rspack")
                        nc.vector.tensor_copy(rspack[64:128, :], oA[64:128, :])
                        nc.vector.tensor_copy(rspack[0:64, :], oB[0:64, :])
                        rr = dpool.tile([128, 512], F32, tag="rr")
                        nc.vector.reciprocal_approx_fast(rr, rspack)
                        rsw = dpool.tile([128, 512], F32, tag="rsw")
                        nc.sync.dma_start(out=rsw[0:64, :], in_=rr[64:128, :])
                        nc.sync.dma_start(out=rsw[64:128, :], in_=rr[0:64, :])
                        nc.vector.tensor_mul(
                            ofin[hp][0:64, qsl], oA[0:64, :], rsw[0:64, :])
                        nc.vector.tensor_mul(
                            ofin[hp][64:128, qsl], oB[64:128, :],
                            rsw[64:128, :])

            # ---- phase C: partial output projection ----
            with tc.tile_pool(name="ps_out", bufs=4, space="PSUM") as ps_out:
                for tt in range(L // 128):
                    tsl = slice(tt * 128, (tt + 1) * 128)
                    for nh in range(2):
                        nsl = slice(nh * 512, (nh + 1) * 512)
                        op = ps_out.tile([128, 512], F32, tag="oproj")
                        nc.tensor.matmul(
                            op, lhsT=ofin[0][:, tsl], rhs=wo_sb[:, 0, nsl],
                            start=True, stop=False,
                        )
                        nc.tensor.matmul(
                            op, lhsT=ofin[1][:, tsl], rhs=wo_sb[:, 1, nsl],
                            start=False, stop=True,
                        )
                        ost = opool.tile([128, 512], BF16, tag="ost")
                        if (tt * 2 + nh) % 2 == 0:
                            nc.vector.tensor_copy(ost, op)
                        else:
                            nc.scalar.copy(ost, op)
                        nc.sync.dma_start(out=partial[tsl, nsl], in_=ost)

    nc.finalize()
    return nc


_NC_CACHE = {}


def _get_nc(kt_k, kt_full):
    key = (kt_k, kt_full)
    if key not in _NC_CACHE:
        _NC_CACHE[key] = _build_nc(kt_k, kt_full)
    return _NC_CACHE[key]


def kernel(q_in, k_in, v_in, kv_pad_mask, Wq, bq, Wk, bk, Wv, bv, Wo, bo,
           _trace=False):
    f32 = np.float32
    bf = ml_dtypes.bfloat16
    f8 = ml_dtypes.float8_e4m3fn
    q_in = np.asarray(q_in, f32)
    k_in = np.asarray(k_in, f32)
    v_in = np.asarray(v_in, f32)
    mask = np.asarray(kv_pad_mask, bool)
    Wq, bq, Wk, bk, Wv, bv, Wo, bo = (
        np.asarray(a, f32) for a in (Wq, bq, Wk, bk, Wv, bv, Wo, bo)
    )

    idx = [np.flatnonzero(~mask[b]) for b in range(B)]
    nv = [len(i) for i in idx]
    kt_k = max(1, max((n + 127) // 128 for n in nv))
    KP = kt_k * 128
    kt_full = min(n // 128 for n in nv)   # tiles < kt_full are all-valid
    nc = _get_nc(kt_k, kt_full)

    # per-batch host prep
    xq8, xk8, xvT, mb, hostconst = {}, {}, {}, {}, {}
    for b in range(B):
        xq8[b] = np.ascontiguousarray(q_in[b].T).astype(f8)
        kc = np.zeros((KP, D), f32)
        kc[:nv[b]] = k_in[b][idx[b]]
        xk8[b] = np.ascontiguousarray(kc.T).astype(f8)
        vc = np.zeros((KP, D), f32)
        vc[:nv[b]] = v_in[b][idx[b]]
        xvT[b] = np.ascontiguousarray(vc.T).astype(bf)
        mrow = np.where(np.arange(KP) < nv[b], 0.0, MASK_BIAS).astype(f32)
        mb[b] = np.ascontiguousarray(mrow.reshape(kt_k, 128).T)
        # 0.1 * uniform-over-valid term is token-independent after out_proj
        n = max(float(nv[b]), 1.0)
        valid = (~mask[b]).astype(f32) / n
        vmean_full = (valid @ v_in[b]) @ Wv.T + bv
        hostconst[b] = bo + EPS_SMOOTH * (vmean_full @ Wo.T)

    in_maps = []
    for core in range(N_CORES):
        b = core // 4
        h0 = (core % 4) * HEADS_PER_CORE
        rows = slice(h0 * DH, h0 * DH + HPC)
        in_maps.append({
            "xqT": xq8[b],
            "xkT": xk8[b],
            "xvT": xvT[b],
            "wq8": np.ascontiguousarray(Wq[rows].T).astype(f8),
            "wk8": np.ascontiguousarray(Wk[rows].T).astype(f8),
            "wv_t": np.ascontiguousarray(Wv[rows].T).astype(bf),
            "wo_t": np.ascontiguousarray(
                (1.0 - EPS_SMOOTH) * Wo[:, rows].T).astype(bf),
            "bq": np.ascontiguousarray(bq[rows].reshape(2, 1, 128)).astype(bf),
            "bk": np.ascontiguousarray(bk[rows].reshape(2, 1, 128)).astype(bf),
            "bv": np.ascontiguousarray(bv[rows].reshape(1, HPC)).astype(bf),
            "mbias": mb[b],
        })

    res = run_bass_kernel_spmd(nc, in_maps, core_ids=list(range(N_CORES)),
                               trace=_trace)
    out = np.zeros((B, L, D), f32)
    for core in range(N_CORES):
        out[core // 4] += res.results[core]["partial"].astype(f32)
    for b in range(B):
        out[b] += hostconst[b][None, :]
    if _trace:
        kernel._last_result = res
    return out
